# revision 49
# baseline (speedup 1.0000x reference)
"""GATv2 (2-layer, N=100, B=8) Trainium2 Bass kernel, 8-core SPMD.

Strategy:
  * The two [10000,10000] f32 lin_n_node matrices dominate HBM traffic.
    edge_att_L = tanh(inv @ WnL.T) depends only on adj_mat, so both big
    matmuls are tensor-parallel sharded over the output dim: core c streams
    WnL columns [c*1250, (c+1)*1250) as fp8e4 (x1024 scale, undone inside
    the tanh), pre-tiled on the host into a partition-major layout so each
    slab DMA is 128 x 25KB fully-contiguous descriptors on the sync ring.
  * inv (= mask * rownorm / adj2) depends only on adj_mat: computed on the
    host, pre-packed fp8 in DoubleRow K-pair layout.
  * Stream matmuls run in fp8 DoubleRow perf mode (2 K-tiles per
    instruction).  K padded 10000 -> 10240 (80 k-tiles), columns 1250 ->
    1264 (stride %16 == 0), stationary batch dim padded 8 -> 16; pads are
    zero so the accumulation is exact.
  * After tanh, an AllToAll hands core c the full [10000] row for batch c.
  * e[i,j] = Wa.tanh(g_i+g_j) is symmetric: only blocks j >= 5*(i//5) are
    computed (~53% of the tanh volume); the lower triangle is filled by a
    PE transpose + predicated copy.
  * Scheduling discipline: big slab DMAs own the sync HWDGE ring;
    latency-critical small transfers (consts, cc_in, ea/e reads) go via
    SWDGE so they never inherit false waits from slab completions on the 8
    shared HWDGE semaphore lanes.  An explicit dep pins the attn-1 PE work
    after the last layer-2 stream matmul: the A2A peer-skew wait (which the
    scheduler's cost model does not see) then always overlaps the layer-2
    stream instead of blocking it in the in-order PE queue.
"""

import sys

for p in ("/opt/trn_rl_repo", "/opt/pypackages"):
    if p not in sys.path:
        sys.path.insert(0, p)

import numpy as np

import concourse.bass as bass
import concourse.mybir as mybir
import concourse.tile as tile
from concourse import bacc
from concourse.bass_utils import run_bass_kernel_spmd
from concourse.tile import add_dep_helper

F32 = mybir.dt.float32
BF16 = mybir.dt.bfloat16
FP8 = mybir.dt.float8e4
U8 = mybir.dt.uint8
AF = mybir.ActivationFunctionType
ALU = mybir.AluOpType
DR = mybir.MatmulPerfMode.DoubleRow

N = 100
N2 = N * N
B = 8
NCORE = 8
SH = N2 // NCORE          # 1250 output columns per core
SHP = 1264                # padded to a multiple of 16 (DoubleRow stride rule)
NKT = 80                  # K padded 10000 -> 10240 = 80 k-tiles of 128
NPAIR = NKT // 2
BP = 16                   # stationary dim padded 8 -> 16 (stride rule)
SLAB = 20                 # k-tiles per slab DMA (4 slabs/layer, ~3.23MB each)
NSLAB = NKT // SLAB
DH = 128                  # hidden dim
INF_ = 64                 # input features
WSCALE = 1024.0           # host scales Wn by this; undone in the tanh
IT_SLICES = [(0, 512), (512, 512), (1024, SHP - 1024)]  # psum bank slices
CHUNK_I = 5               # i-rows per e-chunk

# all f32 constants live in ONE packed [128, CF_TOT] tensor loaded by a
# single DMA: each HWDGE dma_start costs ~0.75us of its sequencer, and a
# late-completing small DMA stalls later same-lane DMAs at issue time
CF_LAYOUT = [
    ("xt", 64, N),            # x[c].T
    ("w_int", 64, DH),
    ("b_in", DH, 1),
    ("wl1t", DH, DH),
    ("mask", N, N),           # has-edge of adj2[c]
    ("id", 128, 128),
    ("w2t", 128, 2 * 2 * DH),     # kt-packed [128, k*m], m=256
    ("b2", DH, 2),
    ("wl2t", 128, 2 * DH),        # m=128
    ("wm1t", 128, 3 * 2 * DH),    # m=256
    ("bm1", DH, 2),
    ("wm2t", 128, 2 * DH),        # m=128
    ("bm2", DH, 1),
    ("wm3t", DH, 2),
    ("bm3", 2, 1),
]
CF_OFF = {}
_o = 0
for _nm, _h, _w in CF_LAYOUT:
    CF_OFF[_nm] = _o
    _o += _w
CF_TOT = _o


def build_nc():
    nc = bacc.Bacc(None, num_devices=NCORE)

    # ---- kernel I/O ----
    wn1p = nc.dram_tensor("wn1p", [128, NKT * SHP], FP8, kind="ExternalInput")
    wn2p = nc.dram_tensor("wn2p", [128, NKT * SHP], FP8, kind="ExternalInput")
    invp = nc.dram_tensor("invp", [128, NKT * BP], FP8, kind="ExternalInput")
    constf = nc.dram_tensor("constf", [128, CF_TOT], F32, kind="ExternalInput")
    constb = nc.dram_tensor("constb", [DH, 2], BF16, kind="ExternalInput")  # wa1|wa2
    masklb = nc.dram_tensor("masklb", [N, N], U8, kind="ExternalInput")  # j < 5*(i//5)
    out_ext = nc.dram_tensor("out", [N, 2], F32, kind="ExternalOutput")

    with tile.TileContext(nc) as tc:
        with (
            tc.tile_pool(name="const", bufs=1) as cpool,
            tc.tile_pool(name="state", bufs=1) as state,
            tc.tile_pool(name="work", bufs=2) as work,
            tc.tile_pool(name="tmpp", bufs=4) as tmpp,
            tc.tile_pool(name="tmp2p", bufs=10) as tmp2p,
            tc.tile_pool(name="ebp", bufs=6) as ebp,
            tc.tile_pool(name="slabs", bufs=5) as slabs,
            tc.tile_pool(name="psbig", bufs=1, space="PSUM") as psbig,
            tc.tile_pool(name="pssm", bufs=2, space="PSUM") as pssm,
            tc.tile_pool(name="dram", bufs=1, space="DRAM") as dram,
        ):
            # ---- 4 coalesced const DMAs on the scalar ring (3us of ACT
            # sequencer at t=0), so the sync ring streams slabs from ~8us ----
            invT_sb = cpool.tile([128, NKT, BP], FP8, name="invT_sb")
            nc.scalar.dma_start(
                invT_sb[:], invp[:].rearrange("p (k b) -> p k b", b=BP)
            )
            cf = cpool.tile([128, CF_TOT], F32, name="cf")
            nc.scalar.dma_start(cf[:], constf[:])
            cb = cpool.tile([DH, 2], BF16, name="cb")
            nc.scalar.dma_start(cb[:], constb[:])
            masklb_sb = cpool.tile([N, N], U8, name="masklb_sb")
            nc.scalar.dma_start(masklb_sb[:], masklb[:])

            def cfv(nm):
                h = dict((n, hh) for n, hh, ww in CF_LAYOUT)[nm]
                w = dict((n, ww) for n, hh, ww in CF_LAYOUT)[nm]
                return cf[0:h, CF_OFF[nm] : CF_OFF[nm] + w]

            def cfk(nm, m, k, lo=0, hi=None):
                """kt-packed weight: column range for k-tile k (+ M slice)."""
                o = CF_OFF[nm] + k * m
                return cf[:, o + lo : o + (m if hi is None else hi)]

            xt_sb = cfv("xt")
            w_int_sb = cfv("w_int")
            b_in_sb = cfv("b_in")
            wl1t_sb = cfv("wl1t")
            mask_sb = cfv("mask")
            id_sb = cfv("id")
            b2_sb = cfv("b2")
            bm1_sb = cfv("bm1")
            bm2_sb = cfv("bm2")
            wm3t_sb = cfv("wm3t")
            bm3_sb = cfv("bm3")
            wa1_sb = cb[:, 0:1]
            wa2_sb = cb[:, 1:2]

            # ---- wn slab stream machinery (sync HWDGE ring only) ----
            _wn_dram = {1: wn1p, 2: wn2p}
            _slab_tiles = {}

            # 6 accumulator banks (DoubleRow rejects col tile_position, and
            # start=True zeroes a full 2KB bank region -> no sharing)
            _accs = {}

            def wn_accs(tag):
                if tag not in _accs:
                    _accs[tag] = [
                        psbig.tile([BP, w], F32, name=f"acc{tag}_{it}")
                        for it, (o, w) in enumerate(IT_SLICES)
                    ]
                return _accs[tag]

            def dma_slab(tag, s):
                t = slabs.tile([128, SLAB, SHP], FP8, name="wns")
                nc.sync.dma_start(
                    t[:],
                    _wn_dram[tag][:, s * SLAB * SHP : (s + 1) * SLAB * SHP]
                    .rearrange("p (k f) -> p k f", f=SHP),
                )
                _slab_tiles[(tag, s)] = t

            def mm_slab(tag, s, first_after=None):
                t = _slab_tiles[(tag, s)]
                accs = wn_accs(tag)
                last = None
                for j in range(0, SLAB, 2):
                    gp = (s * SLAB + j) // 2
                    for it, (o, w) in enumerate(IT_SLICES):
                        last = nc.tensor.matmul(
                            accs[it][:, :w],
                            invT_sb[:, s * SLAB + j : s * SLAB + j + 2, :],
                            t[:, j : j + 2, o : o + w],
                            start=(gp == 0),
                            stop=(gp == NPAIR - 1),
                            perf_mode=DR,
                        )
                        if gp == 0 and first_after is not None:
                            # keep layer-2's PE stream strictly behind
                            # layer-1's (each psum slice is its own
                            # accumulation chain, so pin each chain head)
                            add_dep_helper(last.ins, first_after.ins, sync=True,
                                           reason="wn2 mms after wn1 stream")
                return last

            def a2a_send(tag):
                """tanh(acc/WSCALE) -> cc_in -> AllToAll trigger."""
                accs = wn_accs(tag)
                ea = state.tile([B, SHP], F32, name=f"ea{tag}")
                last_tanh = None
                for it, (o, w) in enumerate(IT_SLICES):
                    last_tanh = nc.scalar.activation(
                        ea[:, o : o + w], accs[it][0:B, :w], AF.Tanh,
                        scale=1.0 / WSCALE,
                    )
                cc_in = dram.tile([B, SH], F32)
                cc_out = dram.tile([B, SH], F32)
                nc.gpsimd.dma_start(cc_in[:], ea[:, 0:SH])
                trig = nc.gpsimd.collective_compute(
                    "AllToAll",
                    ALU.bypass,
                    replica_groups=[list(range(NCORE))],
                    ins=[cc_in[:].opt()],
                    outs=[cc_out[:].opt()],
                )
                return cc_out, trig, last_tanh

            def a2a_read(cc_out, tag):
                ea_ij = state.tile([N, N], F32, name=f"eaij{tag}")
                nc.gpsimd.dma_start(
                    ea_ij[:],
                    cc_out[:].rearrange("b f -> (b f)").rearrange("(i j) -> i j", j=N),
                )
                return ea_ij

            def copy_from_psum(dst_ap, src_ap, engine="vector"):
                if engine == "vector":
                    nc.vector.tensor_copy(dst_ap, src_ap)
                else:
                    nc.scalar.copy(dst_ap, src_ap)

            # ---- layer-1 slab DMAs ----
            for s in range(NSLAB):
                dma_slab(1, s)

            # h_inT = W_in @ x.T + b_in   [128, 100]
            ps = pssm.tile([DH, N], F32, name="ps")
            nc.tensor.matmul(ps[:], w_int_sb[:], xt_sb[:], start=True, stop=True)
            h_inT = state.tile([DH, N], F32, name="h_inT")
            nc.scalar.activation(h_inT[:], ps[:], AF.Identity, bias=b_in_sb[:, 0:1])

            # g1T = Wl1 @ h_inT  [128, 100]
            ps = pssm.tile([DH, N], F32, name="ps")
            nc.tensor.matmul(ps[:], wl1t_sb[:], h_inT[:], start=True, stop=True)
            g1T = state.tile([DH, N], F32, name="g1T")
            copy_from_psum(g1T[:], ps[:])

            def e_chunks(gT, wa_sb, e_dram2d, mm_after=None, write_after=None):
                """e[i,j] = Wa . tanh(g_i + g_j) for j >= 5*(i//5) (symmetric).

                eb copies run on DVE (ACT stays free for tanh), writes go
                via SWDGE so slab-DMA lane ticks never serialize on them.
                The deep eb pool keeps SWDGE write completion (~2.5us) off
                the chunk-to-chunk feedback path, and all eb copies are
                pinned after the last add so the DVE add chain (which gates
                the tanhs and thus ea) never queues behind a PE-paced copy."""
                adds, copies = [], []
                for ci in range(N // CHUNK_I):
                    i0 = ci * CHUNK_I
                    L = N - i0
                    tmp = tmpp.tile([DH, CHUNK_I, N], F32, name="etmp")
                    ad = nc.vector.tensor_tensor(
                        tmp[:, :, :L],
                        gT[:, i0 : i0 + CHUNK_I, None].to_broadcast([DH, CHUNK_I, L]),
                        gT[:, None, i0:N].to_broadcast([DH, CHUNK_I, L]),
                        ALU.add,
                    )
                    adds.append(ad)
                    tmp2 = tmp2p.tile([DH, CHUNK_I, N], BF16, name="etmp2")
                    nc.scalar.activation(tmp2[:, :, :L], tmp[:, :, :L], AF.Tanh)
                    pe = pssm.tile([1, CHUNK_I * N], F32, name="ps")
                    mm = nc.tensor.matmul(
                        pe[:, : CHUNK_I * L], wa_sb[:],
                        tmp2[:, :, :L],
                        start=True, stop=True,
                    )
                    if mm_after is not None:
                        # e2 matmuls stay behind the wn2 stream in the PE
                        # queue (the scheduler's sim has no A2A-skew model)
                        add_dep_helper(mm.ins, mm_after.ins, sync=True,
                                       reason="e2 PE mms after wn2 stream")
                    eb = ebp.tile([1, CHUNK_I * N], F32, name="ebounce")
                    cp = nc.vector.tensor_copy(eb[:, : CHUNK_I * L], pe[:, : CHUNK_I * L])
                    copies.append(cp)
                    wr = nc.gpsimd.dma_start(
                        e_dram2d[i0 : i0 + CHUNK_I, i0:N],
                        eb[0:1, : CHUNK_I * L].rearrange("o (r l) -> o r l", r=CHUNK_I),
                    )
                    if ci == 0 and write_after is not None:
                        add_dep_helper(wr.ins, write_after.ins, sync=True,
                                       reason="e2 writes after cc2 trigger on Pool")
                for ci, cp in enumerate(copies):
                    # keep the DVE add chain >=6 chunks ahead of the PE-paced
                    # copies (ci+6 keeps the dep graph acyclic through the
                    # tmp2/psum buffer reuse edges)
                    ahead = min(ci + 6, len(adds) - 1)
                    add_dep_helper(cp.ins, adds[ahead].ins, sync=True,
                                   reason="eb copies trail the DVE add chain")

            def e_merge(e_dram2d, tag):
                """Read upper-block e, mirror into the lower blocks via PE."""
                e_u = state.tile([N, N], F32, name=f"eij{tag}")
                nc.gpsimd.dma_start(e_u[:], e_dram2d)
                pst = pssm.tile([N, N], F32, name="ps")
                nc.tensor.transpose(pst[:], e_u[:], id_sb[:N, :N])
                e_t = work.tile([N, N], F32, name=f"et{tag}")
                copy_from_psum(e_t[:], pst[:])
                nc.vector.copy_predicated(e_u[:], masklb_sb[:], e_t[:])
                return e_u

            # gnm1 (node-major g1) early: only needs g1T
            def g_node_major(gT, tag):
                psg = pssm.tile([N, DH], F32, name="ps")
                nc.tensor.transpose(psg[:], gT[:], id_sb[:, :])
                gnm = state.tile([N, DH], F32, name=f"gnm{tag}")
                copy_from_psum(gnm[:], psg[:])
                return gnm

            gnm1 = g_node_major(g1T, 1)

            e1_dram = dram.tile([N2], F32)
            e1_dram2d = e1_dram[:].rearrange("(i j) -> i j", j=N)
            e_chunks(g1T, wa1_sb, e1_dram2d)

            # ---- layer-1 stream matmuls + A2A#1 send ----
            last_mm1 = None
            for s in range(NSLAB):
                last_mm1 = mm_slab(1, s)
            cc_out1, _, _ = a2a_send(1)

            # ---- layer-2 slab DMAs (reuse pool bufs) ----
            for s in range(NSLAB):
                dma_slab(2, s)

            e1_ij = e_merge(e1_dram2d, 1)
            ea1_ij = a2a_read(cc_out1, 1)

            # ---- layer-2 stream matmuls + A2A#2 send ----
            last_mm2 = None
            for s in range(NSLAB):
                last_mm2 = mm_slab(2, s, first_after=last_mm1 if s == 0 else None)
            cc_out2, trig2, ea2_tanh = a2a_send(2)

            # =============================================================
            # attention + aggregation (batch side)
            # =============================================================
            def attn_and_aggregate(e_ij, ea_ij, gnm, tag, pin_after=None):
                """softmax(e * ea, 0 off-mask) @ g -> out_T [128, N] psum.

                Reference sets ef=-10000 where ef==0 then softmaxes; with
                exp(-10000)==0 that's the same as exp(ef)*mask."""
                ef = work.tile([N, N], F32, name=f"ef{tag}")
                nc.vector.tensor_mul(out=ef[:], in0=e_ij[:], in1=ea_ij[:])
                aw = work.tile([N, N], F32, name=f"aw{tag}")
                nc.scalar.activation(aw[:], ef[:], AF.Exp)
                nc.vector.tensor_mul(out=aw[:], in0=aw[:], in1=mask_sb[:])
                ssum = work.tile([N, 1], F32, name=f"ssum{tag}")
                nc.vector.tensor_reduce(ssum[:], aw[:], axis=mybir.AxisListType.X, op=ALU.add)
                rsum = work.tile([N, 1], F32, name=f"rsum{tag}")
                nc.vector.reciprocal(rsum[:], ssum[:])
                nc.vector.tensor_scalar_mul(aw[:], aw[:], rsum[:, 0:1])
                # aT via PE transpose
                pst = pssm.tile([N, N], F32, name="ps")
                nc.tensor.transpose(pst[:], aw[:], id_sb[:N, :N])
                awT = work.tile([N, N], F32, name=f"awT{tag}")
                copy_from_psum(awT[:], pst[:])
                # res_T = g.T @ a.T : lhsT = g node-major [j, f], rhs = awT [j, i]
                psr = pssm.tile([DH, N], F32, name="ps")
                nc.tensor.matmul(psr[:], gnm[:], awT[:], start=True, stop=True)
                return psr

            psr1 = attn_and_aggregate(e1_ij, ea1_ij, gnm1, 1)
            out1T = state.tile([DH, N], F32, name="out1T")
            nc.scalar.activation(out1T[:], psr1[:], AF.Tanh)

            # o1T = tanh(W2 @ [out1; h_in] + b2), M split in 2 halves
            o1T = []
            for mh in range(2):
                pso = pssm.tile([DH, N], F32, name="ps")
                lo, hi = mh * DH, (mh + 1) * DH
                nc.tensor.matmul(pso[:], cfk("w2t", 2 * DH, 0, lo, hi), out1T[:], start=True, stop=False)
                nc.tensor.matmul(pso[:], cfk("w2t", 2 * DH, 1, lo, hi), h_inT[:], start=False, stop=True)
                t = state.tile([DH, N], F32, name=f"o1T_{mh}")
                nc.scalar.activation(t[:], pso[:], AF.Tanh, bias=b2_sb[:, mh : mh + 1])
                o1T.append(t)

            # g2T = Wl2 @ o1T  (K = 256)
            psg2 = pssm.tile([DH, N], F32, name="ps")
            nc.tensor.matmul(psg2[:], cfk("wl2t", DH, 0), o1T[0][:], start=True, stop=False)
            nc.tensor.matmul(psg2[:], cfk("wl2t", DH, 1), o1T[1][:], start=False, stop=True)
            g2T = state.tile([DH, N], F32, name="g2T")
            copy_from_psum(g2T[:], psg2[:])

            e2_dram = dram.tile([N2], F32)
            e2_dram2d = e2_dram[:].rearrange("(i j) -> i j", j=N)
            e_chunks(g2T, wa2_sb, e2_dram2d, mm_after=last_mm2, write_after=trig2)
            gnm2 = g_node_major(g2T, 2)
            e2_ij = e_merge(e2_dram2d, 2)

            ea2_ij = a2a_read(cc_out2, 2)

            psr2 = attn_and_aggregate(e2_ij, ea2_ij, gnm2, 2)
            out2T = state.tile([DH, N], F32, name="out2T")
            nc.scalar.activation(out2T[:], psr2[:], AF.Tanh)

            # MLP: q1 = relu(Wm1 @ [out2; o1] + bm1)  (K=384, M=256)
            o2T_parts = [out2T, o1T[0], o1T[1]]
            q1T = []
            for mh in range(2):
                psq = pssm.tile([DH, N], F32, name="ps")
                lo, hi = mh * DH, (mh + 1) * DH
                for kt in range(3):
                    nc.tensor.matmul(
                        psq[:], cfk("wm1t", 2 * DH, kt, lo, hi), o2T_parts[kt][:],
                        start=(kt == 0), stop=(kt == 2),
                    )
                t = state.tile([DH, N], F32, name=f"q1T_{mh}")
                nc.scalar.activation(t[:], psq[:], AF.Relu, bias=bm1_sb[:, mh : mh + 1])
                q1T.append(t)

            # q2 = relu(Wm2 @ q1 + bm2)  (K=256, M=128)
            psq2 = pssm.tile([DH, N], F32, name="ps")
            nc.tensor.matmul(psq2[:], cfk("wm2t", DH, 0), q1T[0][:], start=True, stop=False)
            nc.tensor.matmul(psq2[:], cfk("wm2t", DH, 1), q1T[1][:], start=False, stop=True)
            q2T = state.tile([DH, N], F32, name="q2T")
            nc.scalar.activation(q2T[:], psq2[:], AF.Relu, bias=bm2_sb[:, 0:1])

            # q3 = Wm3 @ q2 + bm3  [2, 100]
            psq3 = pssm.tile([2, N], F32, name="ps")
            nc.tensor.matmul(psq3[:], wm3t_sb[:], q2T[:], start=True, stop=True)
            q3T = state.tile([2, N], F32, name="q3T")
            nc.scalar.activation(q3T[:], psq3[:], AF.Identity, bias=bm3_sb[:, 0:1])

            # transpose -> [100, 2], softmax over classes (free dim)
            psf = pssm.tile([N, 2], F32, name="ps")
            nc.tensor.transpose(psf[:], q3T[:], id_sb[:2, :2])
            qf = work.tile([N, 2], F32, name="qf")
            copy_from_psum(qf[:], psf[:])
            fm = work.tile([N, 1], F32, name="fm")
            nc.vector.tensor_reduce(fm[:], qf[:], axis=mybir.AxisListType.X,
                                    op=ALU.max, negate=True)
            pf = work.tile([N, 2], F32, name="pf")
            nc.scalar.activation(pf[:], qf[:], AF.Exp, bias=fm[:, 0:1])
            sf = work.tile([N, 1], F32, name="sf")
            nc.vector.tensor_reduce(sf[:], pf[:], axis=mybir.AxisListType.X, op=ALU.add)
            rf = work.tile([N, 1], F32, name="rf")
            nc.vector.reciprocal(rf[:], sf[:])
            outp = work.tile([N, 2], F32, name="outp")
            nc.vector.tensor_scalar_mul(outp[:], pf[:], rf[:, 0:1])
            nc.scalar.dma_start(out_ext[:], outp[:])

    nc.compile()
    return nc


_NC_CACHE = None


def _get_nc():
    global _NC_CACHE
    if _NC_CACHE is None:
        _NC_CACHE = build_nc()
    return _NC_CACHE


def _pack_inv(adj):
    """Host-side inv + per-batch edge masks (reference semantics, f32)."""
    eye = np.eye(N, dtype=np.float32)
    withinf = np.where(adj == 0, np.inf, adj)
    dmin = withinf.min(axis=2).astype(np.float32) / 2
    adj2 = adj + dmin[:, :, None] * eye
    norm = np.maximum(
        np.sqrt((adj2.astype(np.float32) ** 2).sum(axis=2, keepdims=True)), 1e-12
    ).astype(np.float32)
    adj_n = (adj2 / norm).astype(np.float32)
    has = adj_n != 0
    inv = np.where(has, 1.0 / np.where(has, adj_n, 1.0), 0.0).astype(np.float32)
    return inv.reshape(B, N2), has


def kernel(x, adj_mat, W_in, b_in, Wl1, Wa1, Wn1, W2, b2, Wl2, Wa2, Wn2,
           Wm1, bm1, Wm2, bm2, Wm3, bm3, _trace=False, _trace_kwargs=None):
    import ml_dtypes
    E4 = ml_dtypes.float8_e4m3
    BF = ml_dtypes.bfloat16

    x = np.asarray(x, dtype=np.float32)
    adj = np.asarray(adj_mat, dtype=np.float32)

    invf, has = _pack_inv(adj)

    # invp [128, NKT*BP]: invp[p, kt*BP + b] = inv[b, kt*128 + p] (0 padded)
    invpad = np.zeros((B, NKT * 128), np.float32)
    invpad[:, :N2] = invf
    invkp = invpad.reshape(B, NKT, 128).transpose(2, 1, 0)  # [128, NKT, B]
    invp_np = np.zeros((128, NKT, BP), np.float32)
    invp_np[:, :, :B] = invkp
    invp_fp8 = np.ascontiguousarray(invp_np.reshape(128, NKT * BP)).astype(E4)

    # block-lower predicate: mirror e from the transpose where j < 5*(i//5)
    ii = np.arange(N)[:, None]
    jj = np.arange(N)[None, :]
    masklb_np = (jj < (ii // CHUNK_I) * CHUNK_I).astype(np.uint8)

    def pack_wn(Wn, c):
        # rhs[p, kt, f] = WSCALE * Wn[c*SH + f, kt*128 + p]
        R = np.asarray(Wn, np.float32)[c * SH : (c + 1) * SH, :]  # [SH, N2]
        blk = np.zeros((NKT * 128, SHP), np.float32)
        blk[:N2, :SH] = R.T * WSCALE
        pk = blk.reshape(NKT, 128, SHP).transpose(1, 0, 2)  # [128, NKT, SHP]
        return np.ascontiguousarray(pk.reshape(128, NKT * SHP)).astype(E4)

    def pack_kt(W):
        # [kt*128, m] -> partition-major [128, kt*m] (contiguous per line)
        WT = np.ascontiguousarray(np.asarray(W, np.float32).T)
        kt, m = WT.shape[0] // 128, WT.shape[1]
        return np.ascontiguousarray(
            WT.reshape(kt, 128, m).transpose(1, 0, 2).reshape(128, kt * m)
        )

    # shared f32 const pack (per-core entries filled below)
    cf_base = np.zeros((128, CF_TOT), np.float32)

    def cf_put(arr, nm, name):
        h, w = dict((n, (hh, ww)) for n, hh, ww in CF_LAYOUT)[name]
        a = np.asarray(nm, np.float32).reshape(h, w)
        arr[:h, CF_OFF[name] : CF_OFF[name] + w] = a

    cf_put(cf_base, np.asarray(W_in, np.float32).T, "w_int")
    cf_put(cf_base, np.asarray(b_in, np.float32).reshape(DH, 1), "b_in")
    cf_put(cf_base, np.asarray(Wl1, np.float32).T, "wl1t")
    cf_put(cf_base, np.eye(128, dtype=np.float32), "id")
    cf_put(cf_base, pack_kt(W2), "w2t")
    cf_put(cf_base, np.asarray(b2, np.float32).reshape(2, DH).T, "b2")
    cf_put(cf_base, pack_kt(Wl2), "wl2t")
    cf_put(cf_base, pack_kt(Wm1), "wm1t")
    cf_put(cf_base, np.asarray(bm1, np.float32).reshape(2, DH).T, "bm1")
    cf_put(cf_base, pack_kt(Wm2), "wm2t")
    cf_put(cf_base, np.asarray(bm2, np.float32).reshape(DH, 1), "bm2")
    cf_put(cf_base, np.asarray(Wm3, np.float32).T, "wm3t")
    cf_put(cf_base, np.asarray(bm3, np.float32).reshape(2, 1), "bm3")

    cb_np = np.zeros((DH, 2), np.float32)
    cb_np[:, 0] = np.asarray(Wa1, np.float32).reshape(DH)
    cb_np[:, 1] = np.asarray(Wa2, np.float32).reshape(DH)
    cb_bf = np.ascontiguousarray(cb_np.astype(BF))

    common = {
        "invp": invp_fp8,
        "masklb": masklb_np,
        "constb": cb_bf,
    }
    in_maps = []
    for c in range(NCORE):
        m = dict(common)
        cfc = cf_base.copy()
        cf_put(cfc, x[c].T, "xt")
        cf_put(cfc, has[c].astype(np.float32), "mask")
        m["constf"] = np.ascontiguousarray(cfc)
        m["wn1p"] = pack_wn(Wn1, c)
        m["wn2p"] = pack_wn(Wn2, c)
        in_maps.append(m)

    nc = _get_nc()
    kw = {}
    if _trace:
        kw["trace"] = True
        if _trace_kwargs:
            kw.update(_trace_kwargs)
    res = run_bass_kernel_spmd(nc, in_maps, core_ids=list(range(NCORE)), **kw)
    out = np.stack([res.results[c]["out"] for c in range(NCORE)], axis=0)
    if _trace:
        kernel._last_results = res
    return out


# revision 52
# speedup vs baseline: 1.0997x; 1.0997x over previous
"""GATv2 (2-layer, N=100, B=8) Trainium2 Bass kernel, 8-core SPMD.

Strategy:
  * The two [10000,10000] f32 lin_n_node matrices dominate HBM traffic.
    edge_att_L = tanh(inv @ WnL.T) depends only on adj_mat, so both big
    matmuls are tensor-parallel sharded over the output dim: core c streams
    WnL columns [c*1250, (c+1)*1250) as fp8e4 (x1024 scale, undone inside
    the tanh), pre-tiled on the host into a partition-major layout so each
    slab DMA is 128 x 25KB fully-contiguous descriptors on the sync ring.
  * inv (= mask * rownorm / adj2) depends only on adj_mat: computed on the
    host, pre-packed fp8 in DoubleRow K-pair layout.
  * Stream matmuls run in fp8 DoubleRow perf mode (2 K-tiles per
    instruction).  K padded 10000 -> 10240 (80 k-tiles), columns 1250 ->
    1264 (stride %16 == 0), stationary batch dim padded 8 -> 16; pads are
    zero so the accumulation is exact.
  * After tanh, an AllToAll hands core c the full [10000] row for batch c.
  * e[i,j] = Wa.tanh(g_i+g_j) is symmetric: only blocks j >= 5*(i//5) are
    computed (~53% of the tanh volume); the lower triangle is filled by a
    PE transpose + predicated copy.
  * Scheduling discipline: big slab DMAs own the sync HWDGE ring;
    latency-critical small transfers (consts, cc_in, ea/e reads) go via
    SWDGE so they never inherit false waits from slab completions on the 8
    shared HWDGE semaphore lanes.  An explicit dep pins the attn-1 PE work
    after the last layer-2 stream matmul: the A2A peer-skew wait (which the
    scheduler's cost model does not see) then always overlaps the layer-2
    stream instead of blocking it in the in-order PE queue.
"""

import sys

for p in ("/opt/trn_rl_repo", "/opt/pypackages"):
    if p not in sys.path:
        sys.path.insert(0, p)

import numpy as np

import concourse.bass as bass
import concourse.mybir as mybir
import concourse.tile as tile
from concourse import bacc
from concourse.bass_utils import run_bass_kernel_spmd
from concourse.tile import add_dep_helper

F32 = mybir.dt.float32
BF16 = mybir.dt.bfloat16
FP8 = mybir.dt.float8e4
U8 = mybir.dt.uint8
AF = mybir.ActivationFunctionType
ALU = mybir.AluOpType
DR = mybir.MatmulPerfMode.DoubleRow

N = 100
N2 = N * N
B = 8
NCORE = 8
SH = N2 // NCORE          # 1250 output columns per core
SHP = 1264                # padded to a multiple of 16 (DoubleRow stride rule)
NKT = 80                  # K padded 10000 -> 10240 = 80 k-tiles of 128
NPAIR = NKT // 2
BP = 16                   # stationary dim padded 8 -> 16 (stride rule)
SLAB = 20                 # k-tiles per slab DMA (4 slabs/layer, ~3.23MB each)
NSLAB = NKT // SLAB
DH = 128                  # hidden dim
INF_ = 64                 # input features
WSCALE = 1024.0           # host scales Wn by this; undone in the tanh
IT_SLICES = [(0, 512), (512, 512), (1024, SHP - 1024)]  # psum bank slices
CHUNK_I = 5               # i-rows per e-chunk

# all f32 constants live in ONE packed [128, CF_TOT] tensor loaded by a
# single DMA: each HWDGE dma_start costs ~0.75us of its sequencer, and a
# late-completing small DMA stalls later same-lane DMAs at issue time
CF_LAYOUT = [
    ("xt", 64, N),            # x[c].T
    ("w_int", 64, DH),
    ("b_in", DH, 1),
    ("wl1t", DH, DH),
    ("mask", N, N),           # has-edge of adj2[c]
    ("id", 128, 128),
    ("w2t", 128, 2 * 2 * DH),     # kt-packed [128, k*m], m=256
    ("b2", DH, 2),
    ("wl2t", 128, 2 * DH),        # m=128
    ("wm1t", 128, 3 * 2 * DH),    # m=256
    ("bm1", DH, 2),
    ("wm2t", 128, 2 * DH),        # m=128
    ("bm2", DH, 1),
    ("wm3t", DH, 2),
    ("bm3", 2, 1),
]
CF_OFF = {}
_o = 0
for _nm, _h, _w in CF_LAYOUT:
    CF_OFF[_nm] = _o
    _o += _w
CF_TOT = _o


def build_nc():
    nc = bacc.Bacc(None, num_devices=NCORE)

    # ---- kernel I/O ----
    wn1p = nc.dram_tensor("wn1p", [128, NKT * SHP], FP8, kind="ExternalInput")
    wn2p = nc.dram_tensor("wn2p", [128, NKT * SHP], FP8, kind="ExternalInput")
    invp = nc.dram_tensor("invp", [128, NKT * BP], FP8, kind="ExternalInput")
    constf = nc.dram_tensor("constf", [128, CF_TOT], F32, kind="ExternalInput")
    constb = nc.dram_tensor("constb", [DH, 2], BF16, kind="ExternalInput")  # wa1|wa2
    masklb = nc.dram_tensor("masklb", [N, N], U8, kind="ExternalInput")  # j < 5*(i//5)
    out_ext = nc.dram_tensor("out", [N, 2], F32, kind="ExternalOutput")

    with tile.TileContext(nc) as tc:
        with (
            tc.tile_pool(name="const", bufs=1) as cpool,
            tc.tile_pool(name="state", bufs=1) as state,
            tc.tile_pool(name="work", bufs=2) as work,
            tc.tile_pool(name="ebig", bufs=1) as ebig,
            tc.tile_pool(name="slabs", bufs=4) as slabs,
            tc.tile_pool(name="psbig", bufs=1, space="PSUM") as psbig,
            tc.tile_pool(name="pssm", bufs=2, space="PSUM") as pssm,
            tc.tile_pool(name="dram", bufs=1, space="DRAM") as dram,
        ):
            # ---- 4 coalesced const DMAs, sync ring, strictly before the
            # slabs: ring FIFO guarantees they transfer first (cross-ring
            # packet arbitration would starve them behind the slab stream) ----
            invT_sb = cpool.tile([128, NKT, BP], FP8, name="invT_sb")
            nc.sync.dma_start(
                invT_sb[:], invp[:].rearrange("p (k b) -> p k b", b=BP)
            )
            cf = cpool.tile([128, CF_TOT], F32, name="cf")
            nc.sync.dma_start(cf[:], constf[:])
            cb = cpool.tile([DH, 2], BF16, name="cb")
            nc.sync.dma_start(cb[:], constb[:])
            masklb_sb = cpool.tile([N, N], U8, name="masklb_sb")
            nc.sync.dma_start(masklb_sb[:], masklb[:])

            def cfv(nm):
                h = dict((n, hh) for n, hh, ww in CF_LAYOUT)[nm]
                w = dict((n, ww) for n, hh, ww in CF_LAYOUT)[nm]
                return cf[0:h, CF_OFF[nm] : CF_OFF[nm] + w]

            def cfk(nm, m, k, lo=0, hi=None):
                """kt-packed weight: column range for k-tile k (+ M slice)."""
                o = CF_OFF[nm] + k * m
                return cf[:, o + lo : o + (m if hi is None else hi)]

            xt_sb = cfv("xt")
            w_int_sb = cfv("w_int")
            b_in_sb = cfv("b_in")
            wl1t_sb = cfv("wl1t")
            mask_sb = cfv("mask")
            id_sb = cfv("id")
            b2_sb = cfv("b2")
            bm1_sb = cfv("bm1")
            bm2_sb = cfv("bm2")
            wm3t_sb = cfv("wm3t")
            bm3_sb = cfv("bm3")
            wa1_sb = cb[:, 0:1]
            wa2_sb = cb[:, 1:2]

            # ---- wn slab stream machinery (sync HWDGE ring only) ----
            _wn_dram = {1: wn1p, 2: wn2p}
            _slab_tiles = {}

            # 6 accumulator banks (DoubleRow rejects col tile_position, and
            # start=True zeroes a full 2KB bank region -> no sharing)
            _accs = {}

            def wn_accs(tag):
                if tag not in _accs:
                    _accs[tag] = [
                        psbig.tile([BP, w], F32, name=f"acc{tag}_{it}")
                        for it, (o, w) in enumerate(IT_SLICES)
                    ]
                return _accs[tag]

            def dma_slab(tag, s):
                t = slabs.tile([128, SLAB, SHP], FP8, name="wns")
                nc.sync.dma_start(
                    t[:],
                    _wn_dram[tag][:, s * SLAB * SHP : (s + 1) * SLAB * SHP]
                    .rearrange("p (k f) -> p k f", f=SHP),
                )
                _slab_tiles[(tag, s)] = t

            def mm_slab(tag, s, first_after=None):
                t = _slab_tiles[(tag, s)]
                accs = wn_accs(tag)
                last = None
                for j in range(0, SLAB, 2):
                    gp = (s * SLAB + j) // 2
                    for it, (o, w) in enumerate(IT_SLICES):
                        last = nc.tensor.matmul(
                            accs[it][:, :w],
                            invT_sb[:, s * SLAB + j : s * SLAB + j + 2, :],
                            t[:, j : j + 2, o : o + w],
                            start=(gp == 0),
                            stop=(gp == NPAIR - 1),
                            perf_mode=DR,
                        )
                        if gp == 0 and first_after is not None:
                            # keep layer-2's PE stream strictly behind
                            # layer-1's (each psum slice is its own
                            # accumulation chain, so pin each chain head)
                            add_dep_helper(last.ins, first_after.ins, sync=True,
                                           reason="wn2 mms after wn1 stream")
                return last

            def a2a_send(tag):
                """tanh(acc/WSCALE) -> cc_in -> AllToAll trigger."""
                accs = wn_accs(tag)
                ea = state.tile([B, SHP], F32, name=f"ea{tag}")
                last_tanh = None
                for it, (o, w) in enumerate(IT_SLICES):
                    last_tanh = nc.scalar.activation(
                        ea[:, o : o + w], accs[it][0:B, :w], AF.Tanh,
                        scale=1.0 / WSCALE,
                    )
                cc_in = dram.tile([B, SH], F32)
                cc_out = dram.tile([B, SH], F32)
                nc.gpsimd.dma_start(cc_in[:], ea[:, 0:SH])
                trig = nc.gpsimd.collective_compute(
                    "AllToAll",
                    ALU.bypass,
                    replica_groups=[list(range(NCORE))],
                    ins=[cc_in[:].opt()],
                    outs=[cc_out[:].opt()],
                )
                return cc_out, trig, last_tanh

            def a2a_read(cc_out, tag):
                ea_ij = state.tile([N, N], F32, name=f"eaij{tag}")
                nc.gpsimd.dma_start(
                    ea_ij[:],
                    cc_out[:].rearrange("b f -> (b f)").rearrange("(i j) -> i j", j=N),
                )
                return ea_ij

            def copy_from_psum(dst_ap, src_ap, engine="vector"):
                if engine == "vector":
                    nc.vector.tensor_copy(dst_ap, src_ap)
                else:
                    nc.scalar.copy(dst_ap, src_ap)

            # ---- layer-1 slab DMAs ----
            for s in range(NSLAB):
                dma_slab(1, s)

            # h_inT = W_in @ x.T + b_in   [128, 100]
            ps = pssm.tile([DH, N], F32, name="ps")
            nc.tensor.matmul(ps[:], w_int_sb[:], xt_sb[:], start=True, stop=True)
            h_inT = state.tile([DH, N], F32, name="h_inT")
            nc.scalar.activation(h_inT[:], ps[:], AF.Identity, bias=b_in_sb[:, 0:1])

            # g1T = Wl1 @ h_inT  [128, 100]
            ps = pssm.tile([DH, N], F32, name="ps")
            nc.tensor.matmul(ps[:], wl1t_sb[:], h_inT[:], start=True, stop=True)
            g1T = state.tile([DH, N], F32, name="g1T")
            copy_from_psum(g1T[:], ps[:])

            # upper-tri chunk layout: (ci, i0, L, flat offset into the big tiles)
            E_OFFS = []
            _eo = 0
            for _ci in range(N // CHUNK_I):
                _i0 = _ci * CHUNK_I
                _L = N - _i0
                E_OFFS.append((_ci, _i0, _L, _eo))
                _eo += CHUNK_I * _L
            TOTE = _eo  # 5250

            def e_chunks(gT, wa_sb, e_dram2d, mm_after=None, write_after=None):
                """e[i,j] = Wa . tanh(g_i + g_j) for j >= 5*(i//5) (symmetric).

                All stages write disjoint regions of single big tiles: no
                buffer cycling, so no WAR edge ever couples the DVE add/tanh
                chain to PE or DMA pacing.  The reduction runs as 11 dense
                512-column matmuls over one contiguous bf16 buffer."""
                tmp_big = ebig.tile([DH, TOTE], F32, name="tmp_big")
                tmp2_big = ebig.tile([DH, TOTE], BF16, name="tmp2_big")
                eb_big = ebig.tile([1, TOTE], F32, name="eb_big")
                for ci, i0, L, off in E_OFFS:
                    dst = tmp_big[:, off : off + CHUNK_I * L].rearrange(
                        "p (a b) -> p a b", b=L
                    )
                    nc.vector.tensor_tensor(
                        dst,
                        gT[:, i0 : i0 + CHUNK_I, None].to_broadcast([DH, CHUNK_I, L]),
                        gT[:, None, i0:N].to_broadcast([DH, CHUNK_I, L]),
                        ALU.add,
                    )
                    nc.scalar.activation(
                        tmp2_big[:, off : off + CHUNK_I * L],
                        tmp_big[:, off : off + CHUNK_I * L],
                        AF.Tanh,
                    )
                for k0 in range(0, TOTE, 512):
                    w = min(512, TOTE - k0)
                    pe = pssm.tile([1, 512], F32, name="ps")
                    mm = nc.tensor.matmul(
                        pe[:, :w], wa_sb, tmp2_big[:, k0 : k0 + w],
                        start=True, stop=True,
                    )
                    if mm_after is not None:
                        # e2 matmuls stay behind the wn2 stream in the PE
                        # queue (the scheduler's sim has no A2A-skew model)
                        add_dep_helper(mm.ins, mm_after.ins, sync=True,
                                       reason="e2 PE mms after wn2 stream")
                    nc.vector.tensor_copy(eb_big[0:1, k0 : k0 + w], pe[:, :w])
                for ci, i0, L, off in E_OFFS:
                    wr = nc.gpsimd.dma_start(
                        e_dram2d[i0 : i0 + CHUNK_I, i0:N],
                        eb_big[0:1, off : off + CHUNK_I * L].rearrange(
                            "o (r l) -> o r l", r=CHUNK_I
                        ),
                    )
                    if ci == 0 and write_after is not None:
                        add_dep_helper(wr.ins, write_after.ins, sync=True,
                                       reason="e2 writes after cc2 trigger on Pool")

            def e_merge(e_dram2d, tag):
                """Read upper-block e, mirror into the lower blocks via PE."""
                e_u = state.tile([N, N], F32, name=f"eij{tag}")
                nc.gpsimd.dma_start(e_u[:], e_dram2d)
                pst = pssm.tile([N, N], F32, name="ps")
                nc.tensor.transpose(pst[:], e_u[:], id_sb[:N, :N])
                e_t = work.tile([N, N], F32, name=f"et{tag}")
                copy_from_psum(e_t[:], pst[:])
                nc.vector.copy_predicated(e_u[:], masklb_sb[:], e_t[:])
                return e_u

            # gnm1 (node-major g1) early: only needs g1T
            def g_node_major(gT, tag):
                psg = pssm.tile([N, DH], F32, name="ps")
                nc.tensor.transpose(psg[:], gT[:], id_sb[:, :])
                gnm = state.tile([N, DH], F32, name=f"gnm{tag}")
                copy_from_psum(gnm[:], psg[:])
                return gnm

            gnm1 = g_node_major(g1T, 1)

            e1_dram = dram.tile([N2], F32)
            e1_dram2d = e1_dram[:].rearrange("(i j) -> i j", j=N)
            e_chunks(g1T, wa1_sb, e1_dram2d)

            # ---- layer-1 stream matmuls + A2A#1 send ----
            last_mm1 = None
            for s in range(NSLAB):
                last_mm1 = mm_slab(1, s)
            cc_out1, _, _ = a2a_send(1)

            # ---- layer-2 slab DMAs (reuse pool bufs) ----
            for s in range(NSLAB):
                dma_slab(2, s)

            e1_ij = e_merge(e1_dram2d, 1)
            ea1_ij = a2a_read(cc_out1, 1)

            # ---- layer-2 stream matmuls + A2A#2 send ----
            last_mm2 = None
            for s in range(NSLAB):
                last_mm2 = mm_slab(2, s, first_after=last_mm1 if s == 0 else None)
            cc_out2, trig2, ea2_tanh = a2a_send(2)

            # =============================================================
            # attention + aggregation (batch side)
            # =============================================================
            def attn_and_aggregate(e_ij, ea_ij, gnm, tag, pin_after=None):
                """softmax(e * ea, 0 off-mask) @ g -> out_T [128, N] psum.

                Reference sets ef=-10000 where ef==0 then softmaxes; with
                exp(-10000)==0 that's the same as exp(ef)*mask."""
                ef = work.tile([N, N], F32, name=f"ef{tag}")
                nc.vector.tensor_mul(out=ef[:], in0=e_ij[:], in1=ea_ij[:])
                aw = work.tile([N, N], F32, name=f"aw{tag}")
                nc.scalar.activation(aw[:], ef[:], AF.Exp)
                nc.vector.tensor_mul(out=aw[:], in0=aw[:], in1=mask_sb[:])
                ssum = work.tile([N, 1], F32, name=f"ssum{tag}")
                nc.vector.tensor_reduce(ssum[:], aw[:], axis=mybir.AxisListType.X, op=ALU.add)
                rsum = work.tile([N, 1], F32, name=f"rsum{tag}")
                nc.vector.reciprocal(rsum[:], ssum[:])
                nc.vector.tensor_scalar_mul(aw[:], aw[:], rsum[:, 0:1])
                # aT via PE transpose
                pst = pssm.tile([N, N], F32, name="ps")
                nc.tensor.transpose(pst[:], aw[:], id_sb[:N, :N])
                awT = work.tile([N, N], F32, name=f"awT{tag}")
                copy_from_psum(awT[:], pst[:])
                # res_T = g.T @ a.T : lhsT = g node-major [j, f], rhs = awT [j, i]
                psr = pssm.tile([DH, N], F32, name="ps")
                nc.tensor.matmul(psr[:], gnm[:], awT[:], start=True, stop=True)
                return psr

            psr1 = attn_and_aggregate(e1_ij, ea1_ij, gnm1, 1)
            out1T = state.tile([DH, N], F32, name="out1T")
            nc.scalar.activation(out1T[:], psr1[:], AF.Tanh)

            # o1T = tanh(W2 @ [out1; h_in] + b2), M split in 2 halves
            o1T = []
            for mh in range(2):
                pso = pssm.tile([DH, N], F32, name="ps")
                lo, hi = mh * DH, (mh + 1) * DH
                nc.tensor.matmul(pso[:], cfk("w2t", 2 * DH, 0, lo, hi), out1T[:], start=True, stop=False)
                nc.tensor.matmul(pso[:], cfk("w2t", 2 * DH, 1, lo, hi), h_inT[:], start=False, stop=True)
                t = state.tile([DH, N], F32, name=f"o1T_{mh}")
                nc.scalar.activation(t[:], pso[:], AF.Tanh, bias=b2_sb[:, mh : mh + 1])
                o1T.append(t)

            # g2T = Wl2 @ o1T  (K = 256)
            psg2 = pssm.tile([DH, N], F32, name="ps")
            nc.tensor.matmul(psg2[:], cfk("wl2t", DH, 0), o1T[0][:], start=True, stop=False)
            nc.tensor.matmul(psg2[:], cfk("wl2t", DH, 1), o1T[1][:], start=False, stop=True)
            g2T = state.tile([DH, N], F32, name="g2T")
            copy_from_psum(g2T[:], psg2[:])

            e2_dram = dram.tile([N2], F32)
            e2_dram2d = e2_dram[:].rearrange("(i j) -> i j", j=N)
            e_chunks(g2T, wa2_sb, e2_dram2d, mm_after=last_mm2, write_after=trig2)
            gnm2 = g_node_major(g2T, 2)
            e2_ij = e_merge(e2_dram2d, 2)

            ea2_ij = a2a_read(cc_out2, 2)

            psr2 = attn_and_aggregate(e2_ij, ea2_ij, gnm2, 2)
            out2T = state.tile([DH, N], F32, name="out2T")
            nc.scalar.activation(out2T[:], psr2[:], AF.Tanh)

            # MLP: q1 = relu(Wm1 @ [out2; o1] + bm1)  (K=384, M=256)
            o2T_parts = [out2T, o1T[0], o1T[1]]
            q1T = []
            for mh in range(2):
                psq = pssm.tile([DH, N], F32, name="ps")
                lo, hi = mh * DH, (mh + 1) * DH
                for kt in range(3):
                    nc.tensor.matmul(
                        psq[:], cfk("wm1t", 2 * DH, kt, lo, hi), o2T_parts[kt][:],
                        start=(kt == 0), stop=(kt == 2),
                    )
                t = state.tile([DH, N], F32, name=f"q1T_{mh}")
                nc.scalar.activation(t[:], psq[:], AF.Relu, bias=bm1_sb[:, mh : mh + 1])
                q1T.append(t)

            # q2 = relu(Wm2 @ q1 + bm2)  (K=256, M=128)
            psq2 = pssm.tile([DH, N], F32, name="ps")
            nc.tensor.matmul(psq2[:], cfk("wm2t", DH, 0), q1T[0][:], start=True, stop=False)
            nc.tensor.matmul(psq2[:], cfk("wm2t", DH, 1), q1T[1][:], start=False, stop=True)
            q2T = state.tile([DH, N], F32, name="q2T")
            nc.scalar.activation(q2T[:], psq2[:], AF.Relu, bias=bm2_sb[:, 0:1])

            # q3 = Wm3 @ q2 + bm3  [2, 100]
            psq3 = pssm.tile([2, N], F32, name="ps")
            nc.tensor.matmul(psq3[:], wm3t_sb[:], q2T[:], start=True, stop=True)
            q3T = state.tile([2, N], F32, name="q3T")
            nc.scalar.activation(q3T[:], psq3[:], AF.Identity, bias=bm3_sb[:, 0:1])

            # transpose -> [100, 2], softmax over classes (free dim)
            psf = pssm.tile([N, 2], F32, name="ps")
            nc.tensor.transpose(psf[:], q3T[:], id_sb[:2, :2])
            qf = work.tile([N, 2], F32, name="qf")
            copy_from_psum(qf[:], psf[:])
            fm = work.tile([N, 1], F32, name="fm")
            nc.vector.tensor_reduce(fm[:], qf[:], axis=mybir.AxisListType.X,
                                    op=ALU.max, negate=True)
            pf = work.tile([N, 2], F32, name="pf")
            nc.scalar.activation(pf[:], qf[:], AF.Exp, bias=fm[:, 0:1])
            sf = work.tile([N, 1], F32, name="sf")
            nc.vector.tensor_reduce(sf[:], pf[:], axis=mybir.AxisListType.X, op=ALU.add)
            rf = work.tile([N, 1], F32, name="rf")
            nc.vector.reciprocal(rf[:], sf[:])
            outp = work.tile([N, 2], F32, name="outp")
            nc.vector.tensor_scalar_mul(outp[:], pf[:], rf[:, 0:1])
            nc.scalar.dma_start(out_ext[:], outp[:])

    nc.compile()
    return nc


_NC_CACHE = None


def _get_nc():
    global _NC_CACHE
    if _NC_CACHE is None:
        _NC_CACHE = build_nc()
    return _NC_CACHE


def _pack_inv(adj):
    """Host-side inv + per-batch edge masks (reference semantics, f32)."""
    eye = np.eye(N, dtype=np.float32)
    withinf = np.where(adj == 0, np.inf, adj)
    dmin = withinf.min(axis=2).astype(np.float32) / 2
    adj2 = adj + dmin[:, :, None] * eye
    norm = np.maximum(
        np.sqrt((adj2.astype(np.float32) ** 2).sum(axis=2, keepdims=True)), 1e-12
    ).astype(np.float32)
    adj_n = (adj2 / norm).astype(np.float32)
    has = adj_n != 0
    inv = np.where(has, 1.0 / np.where(has, adj_n, 1.0), 0.0).astype(np.float32)
    return inv.reshape(B, N2), has


def kernel(x, adj_mat, W_in, b_in, Wl1, Wa1, Wn1, W2, b2, Wl2, Wa2, Wn2,
           Wm1, bm1, Wm2, bm2, Wm3, bm3, _trace=False, _trace_kwargs=None):
    import ml_dtypes
    E4 = ml_dtypes.float8_e4m3
    BF = ml_dtypes.bfloat16

    x = np.asarray(x, dtype=np.float32)
    adj = np.asarray(adj_mat, dtype=np.float32)

    invf, has = _pack_inv(adj)

    # invp [128, NKT*BP]: invp[p, kt*BP + b] = inv[b, kt*128 + p] (0 padded)
    invpad = np.zeros((B, NKT * 128), np.float32)
    invpad[:, :N2] = invf
    invkp = invpad.reshape(B, NKT, 128).transpose(2, 1, 0)  # [128, NKT, B]
    invp_np = np.zeros((128, NKT, BP), np.float32)
    invp_np[:, :, :B] = invkp
    invp_fp8 = np.ascontiguousarray(invp_np.reshape(128, NKT * BP)).astype(E4)

    # block-lower predicate: mirror e from the transpose where j < 5*(i//5)
    ii = np.arange(N)[:, None]
    jj = np.arange(N)[None, :]
    masklb_np = (jj < (ii // CHUNK_I) * CHUNK_I).astype(np.uint8)

    def pack_wn(Wn, c):
        # rhs[p, kt, f] = WSCALE * Wn[c*SH + f, kt*128 + p]
        R = np.asarray(Wn, np.float32)[c * SH : (c + 1) * SH, :]  # [SH, N2]
        blk = np.zeros((NKT * 128, SHP), np.float32)
        blk[:N2, :SH] = R.T * WSCALE
        pk = blk.reshape(NKT, 128, SHP).transpose(1, 0, 2)  # [128, NKT, SHP]
        return np.ascontiguousarray(pk.reshape(128, NKT * SHP)).astype(E4)

    def pack_kt(W):
        # [kt*128, m] -> partition-major [128, kt*m] (contiguous per line)
        WT = np.ascontiguousarray(np.asarray(W, np.float32).T)
        kt, m = WT.shape[0] // 128, WT.shape[1]
        return np.ascontiguousarray(
            WT.reshape(kt, 128, m).transpose(1, 0, 2).reshape(128, kt * m)
        )

    # shared f32 const pack (per-core entries filled below)
    cf_base = np.zeros((128, CF_TOT), np.float32)

    def cf_put(arr, nm, name):
        h, w = dict((n, (hh, ww)) for n, hh, ww in CF_LAYOUT)[name]
        a = np.asarray(nm, np.float32).reshape(h, w)
        arr[:h, CF_OFF[name] : CF_OFF[name] + w] = a

    cf_put(cf_base, np.asarray(W_in, np.float32).T, "w_int")
    cf_put(cf_base, np.asarray(b_in, np.float32).reshape(DH, 1), "b_in")
    cf_put(cf_base, np.asarray(Wl1, np.float32).T, "wl1t")
    cf_put(cf_base, np.eye(128, dtype=np.float32), "id")
    cf_put(cf_base, pack_kt(W2), "w2t")
    cf_put(cf_base, np.asarray(b2, np.float32).reshape(2, DH).T, "b2")
    cf_put(cf_base, pack_kt(Wl2), "wl2t")
    cf_put(cf_base, pack_kt(Wm1), "wm1t")
    cf_put(cf_base, np.asarray(bm1, np.float32).reshape(2, DH).T, "bm1")
    cf_put(cf_base, pack_kt(Wm2), "wm2t")
    cf_put(cf_base, np.asarray(bm2, np.float32).reshape(DH, 1), "bm2")
    cf_put(cf_base, np.asarray(Wm3, np.float32).T, "wm3t")
    cf_put(cf_base, np.asarray(bm3, np.float32).reshape(2, 1), "bm3")

    cb_np = np.zeros((DH, 2), np.float32)
    cb_np[:, 0] = np.asarray(Wa1, np.float32).reshape(DH)
    cb_np[:, 1] = np.asarray(Wa2, np.float32).reshape(DH)
    cb_bf = np.ascontiguousarray(cb_np.astype(BF))

    common = {
        "invp": invp_fp8,
        "masklb": masklb_np,
        "constb": cb_bf,
    }
    in_maps = []
    for c in range(NCORE):
        m = dict(common)
        cfc = cf_base.copy()
        cf_put(cfc, x[c].T, "xt")
        cf_put(cfc, has[c].astype(np.float32), "mask")
        m["constf"] = np.ascontiguousarray(cfc)
        m["wn1p"] = pack_wn(Wn1, c)
        m["wn2p"] = pack_wn(Wn2, c)
        in_maps.append(m)

    nc = _get_nc()
    kw = {}
    if _trace:
        kw["trace"] = True
        if _trace_kwargs:
            kw.update(_trace_kwargs)
    res = run_bass_kernel_spmd(nc, in_maps, core_ids=list(range(NCORE)), **kw)
    out = np.stack([res.results[c]["out"] for c in range(NCORE)], axis=0)
    if _trace:
        kernel._last_results = res
    return out


# revision 56
# speedup vs baseline: 1.2029x; 1.0939x over previous
"""GATv2 (2-layer, N=100, B=8) Trainium2 Bass kernel, 8-core SPMD.

Strategy:
  * The two [10000,10000] f32 lin_n_node matrices dominate HBM traffic.
    edge_att_L = tanh(inv @ WnL.T) depends only on adj_mat, so both big
    matmuls are tensor-parallel sharded over the output dim: core c streams
    WnL columns [c*1250, (c+1)*1250) as fp8e4 (x1024 scale, undone inside
    the tanh), pre-tiled on the host into a partition-major layout so each
    slab DMA is 128 x 25KB fully-contiguous descriptors on the sync ring.
  * inv (= mask * rownorm / adj2) depends only on adj_mat: computed on the
    host, pre-packed fp8 in DoubleRow K-pair layout.
  * Stream matmuls run in fp8 DoubleRow perf mode (2 K-tiles per
    instruction).  K padded 10000 -> 10240 (80 k-tiles), columns 1250 ->
    1264 (stride %16 == 0), stationary batch dim padded 8 -> 16; pads are
    zero so the accumulation is exact.
  * After tanh, an AllToAll hands core c the full [10000] row for batch c.
  * e[i,j] = Wa.tanh(g_i+g_j) is symmetric: only blocks j >= 5*(i//5) are
    computed (~53% of the tanh volume); the lower triangle is filled by a
    PE transpose + predicated copy.
  * Scheduling discipline: big slab DMAs own the sync HWDGE ring;
    latency-critical small transfers (consts, cc_in, ea/e reads) go via
    SWDGE so they never inherit false waits from slab completions on the 8
    shared HWDGE semaphore lanes.  An explicit dep pins the attn-1 PE work
    after the last layer-2 stream matmul: the A2A peer-skew wait (which the
    scheduler's cost model does not see) then always overlaps the layer-2
    stream instead of blocking it in the in-order PE queue.
"""

import sys

for p in ("/opt/trn_rl_repo", "/opt/pypackages"):
    if p not in sys.path:
        sys.path.insert(0, p)

import numpy as np

import concourse.bass as bass
import concourse.mybir as mybir
import concourse.tile as tile
from concourse import bacc
from concourse.bass_utils import run_bass_kernel_spmd
from concourse.tile import add_dep_helper

F32 = mybir.dt.float32
BF16 = mybir.dt.bfloat16
FP8 = mybir.dt.float8e4
U8 = mybir.dt.uint8
AF = mybir.ActivationFunctionType
ALU = mybir.AluOpType
DR = mybir.MatmulPerfMode.DoubleRow

N = 100
N2 = N * N
B = 8
NCORE = 8
SH = N2 // NCORE          # 1250 output columns per core
SHP = 1264                # padded to a multiple of 16 (DoubleRow stride rule)
NKT = 80                  # K padded 10000 -> 10240 = 80 k-tiles of 128
NPAIR = NKT // 2
BP = 16                   # stationary dim padded 8 -> 16 (stride rule)
SLAB = 20                 # k-tiles per slab DMA (4 slabs/layer, ~3.23MB each)
NSLAB = NKT // SLAB
DH = 128                  # hidden dim
INF_ = 64                 # input features
WSCALE = 1024.0           # host scales Wn by this; undone in the tanh
IT_SLICES = [(0, 512), (512, 512), (1024, SHP - 1024)]  # psum bank slices
CHUNK_I = 5               # i-rows per e-chunk

# all f32 constants live in ONE packed [128, CF_TOT] tensor loaded by a
# single DMA: each HWDGE dma_start costs ~0.75us of its sequencer, and a
# late-completing small DMA stalls later same-lane DMAs at issue time
CF_LAYOUT = [
    ("xt", 64, N),            # x[c].T
    ("w_int", 64, DH),
    ("b_in", DH, 1),
    ("wl1t", DH, DH),
    ("mask", N, N),           # has-edge of adj2[c]
    ("id", 128, 128),
    ("w2t", 128, 2 * 2 * DH),     # kt-packed [128, k*m], m=256
    ("b2", DH, 2),
    ("wl2t", 128, 2 * DH),        # m=128
    ("wm1t", 128, 3 * 2 * DH),    # m=256
    ("bm1", DH, 2),
    ("wm2t", 128, 2 * DH),        # m=128
    ("bm2", DH, 1),
    ("wm3t", DH, 2),
    ("bm3", 2, 1),
]
CF_OFF = {}
_o = 0
for _nm, _h, _w in CF_LAYOUT:
    CF_OFF[_nm] = _o
    _o += _w
CF_TOT = _o


def build_nc():
    nc = bacc.Bacc(None, num_devices=NCORE)

    # ---- kernel I/O ----
    wn1p = nc.dram_tensor("wn1p", [128, NKT * SHP], FP8, kind="ExternalInput")
    wn2p = nc.dram_tensor("wn2p", [128, NKT * SHP], FP8, kind="ExternalInput")
    invp = nc.dram_tensor("invp", [128, NKT * BP], FP8, kind="ExternalInput")
    constf = nc.dram_tensor("constf", [128, CF_TOT], F32, kind="ExternalInput")
    constb = nc.dram_tensor("constb", [DH, 2], BF16, kind="ExternalInput")  # wa1|wa2
    masklb = nc.dram_tensor("masklb", [N, N], U8, kind="ExternalInput")  # j < 5*(i//5)
    out_ext = nc.dram_tensor("out", [N, 2], F32, kind="ExternalOutput")

    with tile.TileContext(nc) as tc:
        with (
            tc.tile_pool(name="const", bufs=1) as cpool,
            tc.tile_pool(name="state", bufs=1) as state,
            tc.tile_pool(name="work", bufs=2) as work,
            tc.tile_pool(name="ebig", bufs=1) as ebig,
            tc.tile_pool(name="slabs", bufs=4) as slabs,
            tc.tile_pool(name="psbig", bufs=1, space="PSUM") as psbig,
            tc.tile_pool(name="pssm", bufs=2, space="PSUM") as pssm,
            tc.tile_pool(name="dram", bufs=1, space="DRAM") as dram,
        ):
            # ---- 4 coalesced const DMAs, sync ring, strictly before the
            # slabs: ring FIFO guarantees they transfer first (cross-ring
            # packet arbitration would starve them behind the slab stream) ----
            invT_sb = cpool.tile([128, NKT, BP], FP8, name="invT_sb")
            nc.sync.dma_start(
                invT_sb[:], invp[:].rearrange("p (k b) -> p k b", b=BP)
            )
            cf = cpool.tile([128, CF_TOT], F32, name="cf")
            nc.sync.dma_start(cf[:], constf[:])
            cb = cpool.tile([DH, 2], BF16, name="cb")
            nc.sync.dma_start(cb[:], constb[:])
            masklb_sb = cpool.tile([N, N], U8, name="masklb_sb")
            nc.sync.dma_start(masklb_sb[:], masklb[:])

            def cfv(nm):
                h = dict((n, hh) for n, hh, ww in CF_LAYOUT)[nm]
                w = dict((n, ww) for n, hh, ww in CF_LAYOUT)[nm]
                return cf[0:h, CF_OFF[nm] : CF_OFF[nm] + w]

            def cfk(nm, m, k, lo=0, hi=None):
                """kt-packed weight: column range for k-tile k (+ M slice)."""
                o = CF_OFF[nm] + k * m
                return cf[:, o + lo : o + (m if hi is None else hi)]

            xt_sb = cfv("xt")
            w_int_sb = cfv("w_int")
            b_in_sb = cfv("b_in")
            wl1t_sb = cfv("wl1t")
            mask_sb = cfv("mask")
            id_sb = cfv("id")
            b2_sb = cfv("b2")
            bm1_sb = cfv("bm1")
            bm2_sb = cfv("bm2")
            wm3t_sb = cfv("wm3t")
            bm3_sb = cfv("bm3")
            wa1_sb = cb[:, 0:1]
            wa2_sb = cb[:, 1:2]

            # ---- wn slab stream machinery (sync HWDGE ring only) ----
            _wn_dram = {1: wn1p, 2: wn2p}
            _slab_tiles = {}

            # 6 accumulator banks (DoubleRow rejects col tile_position, and
            # start=True zeroes a full 2KB bank region -> no sharing)
            _accs = {}

            def wn_accs(tag):
                if tag not in _accs:
                    _accs[tag] = [
                        psbig.tile([BP, w], F32, name=f"acc{tag}_{it}")
                        for it, (o, w) in enumerate(IT_SLICES)
                    ]
                return _accs[tag]

            def dma_slab(tag, s):
                t = slabs.tile([128, SLAB, SHP], FP8, name="wns")
                nc.sync.dma_start(
                    t[:],
                    _wn_dram[tag][:, s * SLAB * SHP : (s + 1) * SLAB * SHP]
                    .rearrange("p (k f) -> p k f", f=SHP),
                )
                _slab_tiles[(tag, s)] = t

            def mm_slab(tag, s, first_after=None):
                t = _slab_tiles[(tag, s)]
                accs = wn_accs(tag)
                last = None
                for j in range(0, SLAB, 2):
                    gp = (s * SLAB + j) // 2
                    for it, (o, w) in enumerate(IT_SLICES):
                        last = nc.tensor.matmul(
                            accs[it][:, :w],
                            invT_sb[:, s * SLAB + j : s * SLAB + j + 2, :],
                            t[:, j : j + 2, o : o + w],
                            start=(gp == 0),
                            stop=(gp == NPAIR - 1),
                            perf_mode=DR,
                        )
                        if gp == 0 and first_after is not None:
                            # keep layer-2's PE stream strictly behind
                            # layer-1's (each psum slice is its own
                            # accumulation chain, so pin each chain head)
                            add_dep_helper(last.ins, first_after.ins, sync=True,
                                           reason="wn2 mms after wn1 stream")
                return last

            def a2a_send(tag):
                """tanh(acc/WSCALE) -> cc_in -> AllToAll trigger."""
                accs = wn_accs(tag)
                ea = state.tile([B, SHP], F32, name=f"ea{tag}")
                last_tanh = None
                for it, (o, w) in enumerate(IT_SLICES):
                    last_tanh = nc.scalar.activation(
                        ea[:, o : o + w], accs[it][0:B, :w], AF.Tanh,
                        scale=1.0 / WSCALE,
                    )
                cc_in = dram.tile([B, SH], F32)
                cc_out = dram.tile([B, SH], F32)
                nc.gpsimd.dma_start(cc_in[:], ea[:, 0:SH])
                trig = nc.gpsimd.collective_compute(
                    "AllToAll",
                    ALU.bypass,
                    replica_groups=[list(range(NCORE))],
                    ins=[cc_in[:].opt()],
                    outs=[cc_out[:].opt()],
                )
                return cc_out, trig, last_tanh

            def a2a_read(cc_out, tag):
                ea_ij = state.tile([N, N], F32, name=f"eaij{tag}")
                nc.gpsimd.dma_start(
                    ea_ij[:],
                    cc_out[:].rearrange("b f -> (b f)").rearrange("(i j) -> i j", j=N),
                )
                return ea_ij

            def copy_from_psum(dst_ap, src_ap, engine="vector"):
                if engine == "vector":
                    nc.vector.tensor_copy(dst_ap, src_ap)
                else:
                    nc.scalar.copy(dst_ap, src_ap)

            # ---- layer-1 slab DMAs ----
            for s in range(NSLAB):
                dma_slab(1, s)

            # h_inT = W_in @ x.T + b_in   [128, 100]
            ps = pssm.tile([DH, N], F32, name="ps")
            nc.tensor.matmul(ps[:], w_int_sb[:], xt_sb[:], start=True, stop=True)
            h_inT = state.tile([DH, N], F32, name="h_inT")
            nc.scalar.activation(h_inT[:], ps[:], AF.Identity, bias=b_in_sb[:, 0:1])

            # g1T = Wl1 @ h_inT  [128, 100]
            ps = pssm.tile([DH, N], F32, name="ps")
            nc.tensor.matmul(ps[:], wl1t_sb[:], h_inT[:], start=True, stop=True)
            g1T = state.tile([DH, N], F32, name="g1T")
            copy_from_psum(g1T[:], ps[:])

            # upper-tri chunk layout: (ci, i0, L, flat offset into the big tiles)
            E_OFFS = []
            _eo = 0
            for _ci in range(N // CHUNK_I):
                _i0 = _ci * CHUNK_I
                _L = N - _i0
                E_OFFS.append((_ci, _i0, _L, _eo))
                _eo += CHUNK_I * _L
            TOTE = _eo  # 5250

            def e_chunks(gT, wa_sb, e_dram2d, mm_after=None, write_after=None):
                """e[i,j] = Wa . tanh(g_i + g_j) for j >= 5*(i//5) (symmetric).

                All stages write disjoint regions of single big tiles: no
                buffer cycling, so no WAR edge ever couples the DVE add/tanh
                chain to PE or DMA pacing.  The reduction runs as 11 dense
                512-column matmuls over one contiguous bf16 buffer."""
                tmp_big = ebig.tile([DH, TOTE], F32, name="tmp_big")
                tmp2_big = ebig.tile([DH, TOTE], BF16, name="tmp2_big")
                eb_big = ebig.tile([1, TOTE], F32, name="eb_big")
                for ci, i0, L, off in E_OFFS:
                    dst = tmp_big[:, off : off + CHUNK_I * L].rearrange(
                        "p (a b) -> p a b", b=L
                    )
                    nc.vector.tensor_tensor(
                        dst,
                        gT[:, i0 : i0 + CHUNK_I, None].to_broadcast([DH, CHUNK_I, L]),
                        gT[:, None, i0:N].to_broadcast([DH, CHUNK_I, L]),
                        ALU.add,
                    )
                    nc.scalar.activation(
                        tmp2_big[:, off : off + CHUNK_I * L],
                        tmp_big[:, off : off + CHUNK_I * L],
                        AF.Tanh,
                    )
                for k0 in range(0, TOTE, 512):
                    w = min(512, TOTE - k0)
                    pe = pssm.tile([1, 512], F32, name="ps")
                    mm = nc.tensor.matmul(
                        pe[:, :w], wa_sb, tmp2_big[:, k0 : k0 + w],
                        start=True, stop=True,
                    )
                    if mm_after is not None:
                        # e2 matmuls stay behind the wn2 stream in the PE
                        # queue (the scheduler's sim has no A2A-skew model)
                        add_dep_helper(mm.ins, mm_after.ins, sync=True,
                                       reason="e2 PE mms after wn2 stream")
                    nc.vector.tensor_copy(eb_big[0:1, k0 : k0 + w], pe[:, :w])
                for ci, i0, L, off in E_OFFS:
                    wr = nc.gpsimd.dma_start(
                        e_dram2d[i0 : i0 + CHUNK_I, i0:N],
                        eb_big[0:1, off : off + CHUNK_I * L].rearrange(
                            "o (r l) -> o r l", r=CHUNK_I
                        ),
                    )
                    if ci == 0 and write_after is not None:
                        add_dep_helper(wr.ins, write_after.ins, sync=True,
                                       reason="e2 writes after cc2 trigger on Pool")

            def e_merge(e_dram2d, tag, pin_after=None):
                """Read upper-block e, mirror into the lower blocks via PE."""
                e_u = state.tile([N, N], F32, name=f"eij{tag}")
                nc.gpsimd.dma_start(e_u[:], e_dram2d)
                pst = pssm.tile([N, N], F32, name="ps")
                tr = nc.tensor.transpose(pst[:], e_u[:], id_sb[:N, :N])
                if pin_after is not None:
                    # this transpose waits the slow SWDGE e round-trip; keep
                    # it out of the PE queue ahead of the stream matmuls
                    add_dep_helper(tr.ins, pin_after.ins, sync=True,
                                   reason="e merge transpose after wn stream")
                e_t = work.tile([N, N], F32, name=f"et{tag}")
                copy_from_psum(e_t[:], pst[:])
                nc.vector.copy_predicated(e_u[:], masklb_sb[:], e_t[:])
                return e_u

            # gnm1 (node-major g1) early: only needs g1T
            def g_node_major(gT, tag):
                psg = pssm.tile([N, DH], F32, name="ps")
                nc.tensor.transpose(psg[:], gT[:], id_sb[:, :])
                gnm = state.tile([N, DH], F32, name=f"gnm{tag}")
                copy_from_psum(gnm[:], psg[:])
                return gnm

            gnm1 = g_node_major(g1T, 1)

            e1_dram = dram.tile([N2], F32)
            e1_dram2d = e1_dram[:].rearrange("(i j) -> i j", j=N)
            e_chunks(g1T, wa1_sb, e1_dram2d)

            # ---- layer-1 stream matmuls + A2A#1 send ----
            last_mm1 = None
            for s in range(NSLAB):
                last_mm1 = mm_slab(1, s)
            cc_out1, _, _ = a2a_send(1)

            # ---- layer-2 slab DMAs (reuse pool bufs) ----
            for s in range(NSLAB):
                dma_slab(2, s)

            e1_ij = e_merge(e1_dram2d, 1, pin_after=last_mm1)
            ea1_ij = a2a_read(cc_out1, 1)

            # ---- layer-2 stream matmuls + A2A#2 send ----
            last_mm2 = None
            for s in range(NSLAB):
                last_mm2 = mm_slab(2, s, first_after=last_mm1 if s == 0 else None)
            cc_out2, trig2, ea2_tanh = a2a_send(2)

            # =============================================================
            # attention + aggregation (batch side)
            # =============================================================
            def attn_and_aggregate(e_ij, ea_ij, gnm, tag, pin_after=None):
                """softmax(e * ea, 0 off-mask) @ g -> out_T [128, N] psum.

                Reference sets ef=-10000 where ef==0 then softmaxes; with
                exp(-10000)==0 that's the same as exp(ef)*mask."""
                ef = work.tile([N, N], F32, name=f"ef{tag}")
                mul = nc.vector.tensor_mul(out=ef[:], in0=e_ij[:], in1=ea_ij[:])
                if pin_after is not None:
                    # whole attn1 chain (and the e2 chain downstream of it)
                    # stays behind the wn2 stream: the scheduler's sim has no
                    # A2A-skew model and would let these block the PE queue
                    add_dep_helper(mul.ins, pin_after.ins, sync=True,
                                   reason="attn1 after wn2 stream")
                aw = work.tile([N, N], F32, name=f"aw{tag}")
                nc.scalar.activation(aw[:], ef[:], AF.Exp)
                nc.vector.tensor_mul(out=aw[:], in0=aw[:], in1=mask_sb[:])
                ssum = work.tile([N, 1], F32, name=f"ssum{tag}")
                nc.vector.tensor_reduce(ssum[:], aw[:], axis=mybir.AxisListType.X, op=ALU.add)
                rsum = work.tile([N, 1], F32, name=f"rsum{tag}")
                nc.vector.reciprocal(rsum[:], ssum[:])
                nc.vector.tensor_scalar_mul(aw[:], aw[:], rsum[:, 0:1])
                # aT via PE transpose
                pst = pssm.tile([N, N], F32, name="ps")
                nc.tensor.transpose(pst[:], aw[:], id_sb[:N, :N])
                awT = work.tile([N, N], F32, name=f"awT{tag}")
                copy_from_psum(awT[:], pst[:])
                # res_T = g.T @ a.T : lhsT = g node-major [j, f], rhs = awT [j, i]
                psr = pssm.tile([DH, N], F32, name="ps")
                nc.tensor.matmul(psr[:], gnm[:], awT[:], start=True, stop=True)
                return psr

            psr1 = attn_and_aggregate(e1_ij, ea1_ij, gnm1, 1, pin_after=last_mm2)
            out1T = state.tile([DH, N], F32, name="out1T")
            nc.scalar.activation(out1T[:], psr1[:], AF.Tanh)

            # o1T = tanh(W2 @ [out1; h_in] + b2), M split in 2 halves
            o1T = []
            for mh in range(2):
                pso = pssm.tile([DH, N], F32, name="ps")
                lo, hi = mh * DH, (mh + 1) * DH
                nc.tensor.matmul(pso[:], cfk("w2t", 2 * DH, 0, lo, hi), out1T[:], start=True, stop=False)
                nc.tensor.matmul(pso[:], cfk("w2t", 2 * DH, 1, lo, hi), h_inT[:], start=False, stop=True)
                t = state.tile([DH, N], F32, name=f"o1T_{mh}")
                nc.scalar.activation(t[:], pso[:], AF.Tanh, bias=b2_sb[:, mh : mh + 1])
                o1T.append(t)

            # g2T = Wl2 @ o1T  (K = 256)
            psg2 = pssm.tile([DH, N], F32, name="ps")
            nc.tensor.matmul(psg2[:], cfk("wl2t", DH, 0), o1T[0][:], start=True, stop=False)
            nc.tensor.matmul(psg2[:], cfk("wl2t", DH, 1), o1T[1][:], start=False, stop=True)
            g2T = state.tile([DH, N], F32, name="g2T")
            copy_from_psum(g2T[:], psg2[:])

            e2_dram = dram.tile([N2], F32)
            e2_dram2d = e2_dram[:].rearrange("(i j) -> i j", j=N)
            e_chunks(g2T, wa2_sb, e2_dram2d, mm_after=last_mm2, write_after=trig2)
            gnm2 = g_node_major(g2T, 2)
            e2_ij = e_merge(e2_dram2d, 2)

            ea2_ij = a2a_read(cc_out2, 2)

            psr2 = attn_and_aggregate(e2_ij, ea2_ij, gnm2, 2)
            out2T = state.tile([DH, N], F32, name="out2T")
            nc.scalar.activation(out2T[:], psr2[:], AF.Tanh)

            # MLP: q1 = relu(Wm1 @ [out2; o1] + bm1)  (K=384, M=256)
            o2T_parts = [out2T, o1T[0], o1T[1]]
            q1T = []
            for mh in range(2):
                psq = pssm.tile([DH, N], F32, name="ps")
                lo, hi = mh * DH, (mh + 1) * DH
                for kt in range(3):
                    nc.tensor.matmul(
                        psq[:], cfk("wm1t", 2 * DH, kt, lo, hi), o2T_parts[kt][:],
                        start=(kt == 0), stop=(kt == 2),
                    )
                t = state.tile([DH, N], F32, name=f"q1T_{mh}")
                nc.scalar.activation(t[:], psq[:], AF.Relu, bias=bm1_sb[:, mh : mh + 1])
                q1T.append(t)

            # q2 = relu(Wm2 @ q1 + bm2)  (K=256, M=128)
            psq2 = pssm.tile([DH, N], F32, name="ps")
            nc.tensor.matmul(psq2[:], cfk("wm2t", DH, 0), q1T[0][:], start=True, stop=False)
            nc.tensor.matmul(psq2[:], cfk("wm2t", DH, 1), q1T[1][:], start=False, stop=True)
            q2T = state.tile([DH, N], F32, name="q2T")
            nc.scalar.activation(q2T[:], psq2[:], AF.Relu, bias=bm2_sb[:, 0:1])

            # q3 = Wm3 @ q2 + bm3  [2, 100]
            psq3 = pssm.tile([2, N], F32, name="ps")
            nc.tensor.matmul(psq3[:], wm3t_sb[:], q2T[:], start=True, stop=True)
            q3T = state.tile([2, N], F32, name="q3T")
            nc.scalar.activation(q3T[:], psq3[:], AF.Identity, bias=bm3_sb[:, 0:1])

            # transpose -> [100, 2], softmax over classes (free dim)
            psf = pssm.tile([N, 2], F32, name="ps")
            nc.tensor.transpose(psf[:], q3T[:], id_sb[:2, :2])
            qf = work.tile([N, 2], F32, name="qf")
            copy_from_psum(qf[:], psf[:])
            fm = work.tile([N, 1], F32, name="fm")
            nc.vector.tensor_reduce(fm[:], qf[:], axis=mybir.AxisListType.X,
                                    op=ALU.max, negate=True)
            pf = work.tile([N, 2], F32, name="pf")
            nc.scalar.activation(pf[:], qf[:], AF.Exp, bias=fm[:, 0:1])
            sf = work.tile([N, 1], F32, name="sf")
            nc.vector.tensor_reduce(sf[:], pf[:], axis=mybir.AxisListType.X, op=ALU.add)
            rf = work.tile([N, 1], F32, name="rf")
            nc.vector.reciprocal(rf[:], sf[:])
            outp = work.tile([N, 2], F32, name="outp")
            nc.vector.tensor_scalar_mul(outp[:], pf[:], rf[:, 0:1])
            nc.scalar.dma_start(out_ext[:], outp[:])

    nc.compile()
    return nc


_NC_CACHE = None


def _get_nc():
    global _NC_CACHE
    if _NC_CACHE is None:
        _NC_CACHE = build_nc()
    return _NC_CACHE


def _pack_inv(adj):
    """Host-side inv + per-batch edge masks (reference semantics, f32)."""
    eye = np.eye(N, dtype=np.float32)
    withinf = np.where(adj == 0, np.inf, adj)
    dmin = withinf.min(axis=2).astype(np.float32) / 2
    adj2 = adj + dmin[:, :, None] * eye
    norm = np.maximum(
        np.sqrt((adj2.astype(np.float32) ** 2).sum(axis=2, keepdims=True)), 1e-12
    ).astype(np.float32)
    adj_n = (adj2 / norm).astype(np.float32)
    has = adj_n != 0
    inv = np.where(has, 1.0 / np.where(has, adj_n, 1.0), 0.0).astype(np.float32)
    return inv.reshape(B, N2), has


def kernel(x, adj_mat, W_in, b_in, Wl1, Wa1, Wn1, W2, b2, Wl2, Wa2, Wn2,
           Wm1, bm1, Wm2, bm2, Wm3, bm3, _trace=False, _trace_kwargs=None):
    import ml_dtypes
    E4 = ml_dtypes.float8_e4m3
    BF = ml_dtypes.bfloat16

    x = np.asarray(x, dtype=np.float32)
    adj = np.asarray(adj_mat, dtype=np.float32)

    invf, has = _pack_inv(adj)

    # invp [128, NKT*BP]: invp[p, kt*BP + b] = inv[b, kt*128 + p] (0 padded)
    invpad = np.zeros((B, NKT * 128), np.float32)
    invpad[:, :N2] = invf
    invkp = invpad.reshape(B, NKT, 128).transpose(2, 1, 0)  # [128, NKT, B]
    invp_np = np.zeros((128, NKT, BP), np.float32)
    invp_np[:, :, :B] = invkp
    invp_fp8 = np.ascontiguousarray(invp_np.reshape(128, NKT * BP)).astype(E4)

    # block-lower predicate: mirror e from the transpose where j < 5*(i//5)
    ii = np.arange(N)[:, None]
    jj = np.arange(N)[None, :]
    masklb_np = (jj < (ii // CHUNK_I) * CHUNK_I).astype(np.uint8)

    def pack_wn(Wn, c):
        # rhs[p, kt, f] = WSCALE * Wn[c*SH + f, kt*128 + p]
        R = np.asarray(Wn, np.float32)[c * SH : (c + 1) * SH, :]  # [SH, N2]
        blk = np.zeros((NKT * 128, SHP), np.float32)
        blk[:N2, :SH] = R.T * WSCALE
        pk = blk.reshape(NKT, 128, SHP).transpose(1, 0, 2)  # [128, NKT, SHP]
        return np.ascontiguousarray(pk.reshape(128, NKT * SHP)).astype(E4)

    def pack_kt(W):
        # [kt*128, m] -> partition-major [128, kt*m] (contiguous per line)
        WT = np.ascontiguousarray(np.asarray(W, np.float32).T)
        kt, m = WT.shape[0] // 128, WT.shape[1]
        return np.ascontiguousarray(
            WT.reshape(kt, 128, m).transpose(1, 0, 2).reshape(128, kt * m)
        )

    # shared f32 const pack (per-core entries filled below)
    cf_base = np.zeros((128, CF_TOT), np.float32)

    def cf_put(arr, nm, name):
        h, w = dict((n, (hh, ww)) for n, hh, ww in CF_LAYOUT)[name]
        a = np.asarray(nm, np.float32).reshape(h, w)
        arr[:h, CF_OFF[name] : CF_OFF[name] + w] = a

    cf_put(cf_base, np.asarray(W_in, np.float32).T, "w_int")
    cf_put(cf_base, np.asarray(b_in, np.float32).reshape(DH, 1), "b_in")
    cf_put(cf_base, np.asarray(Wl1, np.float32).T, "wl1t")
    cf_put(cf_base, np.eye(128, dtype=np.float32), "id")
    cf_put(cf_base, pack_kt(W2), "w2t")
    cf_put(cf_base, np.asarray(b2, np.float32).reshape(2, DH).T, "b2")
    cf_put(cf_base, pack_kt(Wl2), "wl2t")
    cf_put(cf_base, pack_kt(Wm1), "wm1t")
    cf_put(cf_base, np.asarray(bm1, np.float32).reshape(2, DH).T, "bm1")
    cf_put(cf_base, pack_kt(Wm2), "wm2t")
    cf_put(cf_base, np.asarray(bm2, np.float32).reshape(DH, 1), "bm2")
    cf_put(cf_base, np.asarray(Wm3, np.float32).T, "wm3t")
    cf_put(cf_base, np.asarray(bm3, np.float32).reshape(2, 1), "bm3")

    cb_np = np.zeros((DH, 2), np.float32)
    cb_np[:, 0] = np.asarray(Wa1, np.float32).reshape(DH)
    cb_np[:, 1] = np.asarray(Wa2, np.float32).reshape(DH)
    cb_bf = np.ascontiguousarray(cb_np.astype(BF))

    common = {
        "invp": invp_fp8,
        "masklb": masklb_np,
        "constb": cb_bf,
    }
    in_maps = []
    for c in range(NCORE):
        m = dict(common)
        cfc = cf_base.copy()
        cf_put(cfc, x[c].T, "xt")
        cf_put(cfc, has[c].astype(np.float32), "mask")
        m["constf"] = np.ascontiguousarray(cfc)
        m["wn1p"] = pack_wn(Wn1, c)
        m["wn2p"] = pack_wn(Wn2, c)
        in_maps.append(m)

    nc = _get_nc()
    kw = {}
    if _trace:
        kw["trace"] = True
        if _trace_kwargs:
            kw.update(_trace_kwargs)
    res = run_bass_kernel_spmd(nc, in_maps, core_ids=list(range(NCORE)), **kw)
    out = np.stack([res.results[c]["out"] for c in range(NCORE)], axis=0)
    if _trace:
        kernel._last_results = res
    return out
